# revision 1
# baseline (speedup 1.0000x reference)
"""Two-layer GAT on 8 TRN2 NeuronCores.

Strategy (edge-parallel, dst-sharded):
  - Host planner partitions dst nodes across 8 cores into degree-sorted
    windows of 128 nodes (lanes).  Each node's incoming edges occupy
    "slots" along the SBUF free dim; every window is padded to its max
    per-zone degree.  Per-edge tables rows (h | a_s | a_d) are fetched
    with dma_gather keyed by src.  Softmax (without max-subtraction --
    logits are bounded) and the weighted sum are computed per-lane with
    free-dim reductions; no scatter is ever needed on device.
  - int16 gather indices cap the index range at 32767, so each table is
    addressed in two zones (lo/hi rows) with per-node slot-zones.
  - Between layers one AllGather replicates the produced h1 rows.

Layout of a table row (128 bf16 = 256B):
  [0:96] h (post-linear, pre-attention)  [96:99] a_s  [99:102] a_d
Row 0 and the last row are pad rows: h=0, a_s=-200 (=> exp ~ 0), a_d=0.
"""

import dataclasses
import numpy as np

from concourse import bass, mybir, tile
from concourse.bacc import Bacc
from concourse.bass_utils import run_bass_kernel_spmd
from concourse.masks import make_identity

F32 = mybir.dt.float32
BF16 = mybir.dt.bfloat16
I16 = mybir.dt.int16
AX = mybir.AxisListType
OP = mybir.AluOpType
AF = mybir.ActivationFunctionType


@dataclasses.dataclass
class Cfg:
    N: int = 50000          # real nodes
    H: int = 3
    IN: int = 128
    F1: int = 32            # per-head feats layer1 (concat -> 96)
    F2: int = 32
    CORES: int = 8
    LANES: int = 128
    THR: int = 32766        # node/pos <= THR -> lo zone (idx = v+1 <= 32767)

    @property
    def NH1(self):
        return self.H * self.F1    # 96

    @property
    def NV(self):
        per = self.CORES * self.LANES
        return ((self.N + per - 1) // per) * per

    @property
    def W(self):
        return self.NV // (self.CORES * self.LANES)

    @property
    def NPC(self):
        return self.W * self.LANES


def _layout_for_layer(cfg, dst_s, srckey_s, selfmask_s):
    """Compute slots/zones for one layer.

    dst_s:     [E] dst node ids, sorted ascending (stable)
    srckey_s:  [E] gather key of the src endpoint (node id or perm pos)
    selfmask_s:[E] bool, true iff edge is a self loop (src==dst)
    Returns dict(slot, zone, lo_need, hi_need) -- slot/zone per edge.
    """
    N = cfg.N
    E = len(dst_s)
    zone = (srckey_s > cfg.THR).astype(np.int8)

    # one designated self edge per node -> slot 0 of its zone
    selfpos = np.full(N, -1, np.int64)
    cand = np.nonzero(selfmask_s)[0][::-1]
    selfpos[dst_s[cand]] = cand          # first occurrence wins
    assert (selfpos >= 0).all(), "every node needs a self loop"
    designated = np.zeros(E, bool)
    designated[selfpos] = True

    # ranks within (node, zone) among non-designated edges
    norm_idx = np.nonzero(~designated)[0]
    key = dst_s[norm_idx].astype(np.int64) * 2 + zone[norm_idx]
    order = np.argsort(key, kind="stable")
    ksort = key[order]
    grp_start = np.r_[0, np.nonzero(np.diff(ksort))[0] + 1]
    start_of = np.repeat(grp_start, np.diff(np.r_[grp_start, len(ksort)]))
    rank_sorted = np.arange(len(ksort)) - start_of
    slot = np.empty(E, np.int32)
    slot[norm_idx[order]] = rank_sorted + 1        # slots 1..
    slot[designated] = 0

    nlo_e = np.bincount(dst_s[zone == 0], minlength=N)   # lo edges incl self
    nhi_e = np.bincount(dst_s[zone == 1], minlength=N)
    node_hi = np.zeros(N, bool)
    node_hi[dst_s[designated]] = zone[designated] == 1
    lo_need = nlo_e + node_hi.astype(np.int64)           # +1 pad slot 0 if self is hi
    hi_need = nhi_e + (~node_hi).astype(np.int64)
    return dict(slot=slot, zone=zone, lo_need=lo_need, hi_need=hi_need)


def _windows(cfg, lo_need, hi_need):
    """Sort nodes lex by (lo_need, hi_need); deal to cores; window maxima."""
    N, NV, W = cfg.N, cfg.NV, cfg.W
    order = np.lexsort((hi_need, lo_need))       # [N] node ids by rank
    rank_of = np.empty(N, np.int64)
    rank_of[order] = np.arange(N)
    # rank r -> core r%C, q r//C; window q//LANES, lane q%LANES
    lo_v = np.zeros(NV, np.int64)
    hi_v = np.zeros(NV, np.int64)
    lo_v[: N] = lo_need[order]
    hi_v[: N] = hi_need[order]
    per_w = cfg.CORES * cfg.LANES
    SLO = np.maximum(1, lo_v.reshape(W, per_w).max(1)).astype(np.int64)
    SHI = np.maximum(1, hi_v.reshape(W, per_w).max(1)).astype(np.int64)
    return order, rank_of, SLO, SHI


def _fill_idx(cfg, rank_of, SLO, SHI, dst_s, slot, zone, idxval, pad_hi_idx):
    """Build per-core wrapped idx arrays [128, C] int16.

    Column layout: for w in range(W): [8*SLO[w] lo cols][8*SHI[w] hi cols].
    """
    W, LANES, CORES = cfg.W, cfg.LANES, cfg.CORES
    colw = 8 * (SLO + SHI)
    col_base = np.r_[0, np.cumsum(colw)][:-1]
    C = int(colw.sum())
    arrs = np.zeros((CORES, 16, C), np.int16)
    # prefill hi zones with their pad idx (lo pad idx is 0 already)
    for w in range(W):
        h0 = col_base[w] + 8 * SLO[w]
        arrs[:, :, h0: h0 + 8 * SHI[w]] = pad_hi_idx

    rank = rank_of[dst_s]
    core = rank % CORES
    q = rank // CORES
    lane = q % LANES
    wno = q // LANES
    base = col_base[wno] + np.where(zone == 1, 8 * SLO[wno], 0)
    i = slot.astype(np.int64) * LANES + lane
    col = base + i // 16
    row = i % 16
    v = idxval.astype(np.int16)
    arrs[core, row, col] = v
    out = np.zeros((CORES, 128, C), np.int16)
    for k in range(8):
        out[:, k * 16:(k + 1) * 16, :] = arrs
    return out, col_base


def make_plan(cfg, edge_index):
    N = cfg.N
    ei = np.asarray(edge_index)
    src = np.concatenate([ei[0], np.arange(N, dtype=np.int64)]).astype(np.int64)
    dst = np.concatenate([ei[1], np.arange(N, dtype=np.int64)]).astype(np.int64)
    eo = np.argsort(dst, kind="stable")
    src_s, dst_s = src[eo], dst[eo]
    selfmask_s = src_s == dst_s
    deg = np.bincount(dst_s, minlength=N)

    # ---- layer 1 (gather key = original node id)
    l1 = _layout_for_layer(cfg, dst_s, src_s, selfmask_s)
    order1, rank1, SLO1, SHI1 = _windows(cfg, l1["lo_need"], l1["hi_need"])
    idxval1 = np.where(l1["zone"] == 0, src_s + 1, src_s - (cfg.THR + 1))
    pad_hi1 = cfg.N - cfg.THR - 1             # row N+1 is pad2; hi base row THR+2
    idx1, _ = _fill_idx(cfg, rank1, SLO1, SHI1, dst_s, l1["slot"], l1["zone"],
                        idxval1, pad_hi1)

    # pos1: node -> row in h1_all  (core-major concat of per-core window rows)
    core1 = rank1 % cfg.CORES
    q1 = rank1 // cfg.CORES
    pos1 = core1 * cfg.NPC + q1               # [N]

    # ---- layer 2 (gather key = pos1 of src)
    key2_s = pos1[src_s]
    l2 = _layout_for_layer(cfg, dst_s, key2_s, selfmask_s)
    order2, rank2, SLO2, SHI2 = _windows(cfg, l2["lo_need"], l2["hi_need"])
    idxval2 = np.where(l2["zone"] == 0, key2_s + 1, key2_s - (cfg.THR + 1))
    pad_hi2 = cfg.NV - cfg.THR - 1
    idx2, _ = _fill_idx(cfg, rank2, SLO2, SHI2, dst_s, l2["slot"], l2["zone"],
                        idxval2, pad_hi2)

    core2 = rank2 % cfg.CORES
    q2 = rank2 // cfg.CORES
    return dict(cfg=cfg, SLO1=SLO1, SHI1=SHI1, SLO2=SLO2, SHI2=SHI2,
                idx1=idx1, idx2=idx2, core2=core2, q2=q2)


# --------------------------------------------------------------------------
# device program
# --------------------------------------------------------------------------

def build_program(plan):
    cfg: Cfg = plan["cfg"]
    N, NV, W, NPC = cfg.N, cfg.NV, cfg.W, cfg.NPC
    NH = cfg.NH1                      # 96
    SLO1, SHI1 = plan["SLO1"], plan["SHI1"]
    SLO2, SHI2 = plan["SLO2"], plan["SHI2"]
    C1 = idx_cols(SLO1, SHI1)
    C2 = idx_cols(SLO2, SHI2)
    T1R = N + 2                        # table1 rows (pad + N + pad)
    T2R = NV + 2
    LO_ROWS = cfg.THR + 2              # rows [0, THR+2): pad + keys 0..THR
    nc = Bacc("TRN2", target_bir_lowering=False, debug=False,
              num_devices=cfg.CORES)

    t_x = nc.dram_tensor("x", [N, cfg.IN], F32, kind="ExternalInput")
    t_W1 = nc.dram_tensor("W1", [cfg.IN, NH], F32, kind="ExternalInput")
    t_b1 = nc.dram_tensor("b1", [NH], F32, kind="ExternalInput")
    t_as1 = nc.dram_tensor("att_src1", [cfg.H, cfg.F1], F32, kind="ExternalInput")
    t_ad1 = nc.dram_tensor("att_dst1", [cfg.H, cfg.F1], F32, kind="ExternalInput")
    t_W2 = nc.dram_tensor("W2", [NH, NH], F32, kind="ExternalInput")
    t_b2 = nc.dram_tensor("b2", [cfg.F2], F32, kind="ExternalInput")
    t_as2 = nc.dram_tensor("att_src2", [cfg.H, cfg.F2], F32, kind="ExternalInput")
    t_ad2 = nc.dram_tensor("att_dst2", [cfg.H, cfg.F2], F32, kind="ExternalInput")
    t_fW1 = nc.dram_tensor("ffW1", [cfg.F2, cfg.F2 // 2], F32, kind="ExternalInput")
    t_fb1 = nc.dram_tensor("ffb1", [cfg.F2 // 2], F32, kind="ExternalInput")
    t_fW2 = nc.dram_tensor("ffW2", [cfg.F2 // 2, 2], F32, kind="ExternalInput")
    t_fb2 = nc.dram_tensor("ffb2", [2], F32, kind="ExternalInput")
    t_idx1 = nc.dram_tensor("idx1", [128, C1], I16, kind="ExternalInput")
    t_idx2 = nc.dram_tensor("idx2", [128, C2], I16, kind="ExternalInput")
    t_out = nc.dram_tensor("out", [NPC, 2], F32, kind="ExternalOutput")

    table1 = nc.dram_tensor("table1", [T1R, 128], BF16)
    table2 = nc.dram_tensor("table2", [T2R, 128], BF16)
    h1_shard = nc.dram_tensor("h1_shard", [NPC, NH], BF16)
    h1_all = nc.dram_tensor("h1_all", [NV, NH], BF16, addr_space="Shared")

    with tile.TileContext(nc, num_cores=cfg.CORES) as tc:
        with tc.tile_pool(name="const", bufs=1) as cp:
            idf = cp.tile([128, 128], F32)
            make_identity(nc, idf[:])
            ident = cp.tile([128, 128], BF16)
            nc.vector.tensor_copy(out=ident[:], in_=idf[:])

            # --- weights
            W1sb = cp.tile([128, NH], BF16)
            nc.gpsimd.dma_start(out=W1sb[:], in_=t_W1[:])
            W2sb = cp.tile([NH, NH], BF16)
            nc.gpsimd.dma_start(out=W2sb[:], in_=t_W2[:])
            fW1b = cp.tile([cfg.F2, cfg.F2 // 2], BF16)
            nc.gpsimd.dma_start(out=fW1b[:], in_=t_fW1[:])
            fW2b = cp.tile([cfg.F2 // 2, 2], BF16)
            nc.gpsimd.dma_start(out=fW2b[:], in_=t_fW2[:])

            A1s = cp.tile([NH, cfg.H], BF16)
            A1d = cp.tile([NH, cfg.H], BF16)
            A2s = cp.tile([NH, cfg.H], BF16)
            A2d = cp.tile([NH, cfg.H], BF16)
            for (a_t, t_a, Ff) in ((A1s, t_as1, cfg.F1), (A1d, t_ad1, cfg.F1),
                                   (A2s, t_as2, cfg.F2), (A2d, t_ad2, cfg.F2)):
                nc.vector.memset(a_t[:], 0.0)
                for hd in range(cfg.H):
                    nc.gpsimd.dma_start(
                        out=a_t[hd * Ff:(hd + 1) * Ff, hd:hd + 1],
                        in_=t_a[hd, :, None])

            with tc.tile_pool(name="setup_ps", bufs=1, space="PSUM") as sps:
                # W1ext = [W1 | W1@A1s | W1@A1d]
                pT = sps.tile([128, 128], BF16)
                nc.tensor.transpose(out=pT[0:NH, :], in_=W1sb[:], identity=ident[:])
                W1T = cp.tile([NH, 128], BF16)
                nc.vector.tensor_copy(out=W1T[:], in_=pT[0:NH, :])
                W1ext = cp.tile([128, 128], BF16)
                nc.vector.memset(W1ext[:], 0.0)
                nc.vector.tensor_copy(out=W1ext[:, 0:NH], in_=W1sb[:])
                pA = sps.tile([128, cfg.H], F32)
                nc.tensor.matmul(out=pA[:], lhsT=W1T[:], rhs=A1s[:], start=True, stop=True)
                nc.vector.tensor_copy(out=W1ext[:, NH:NH + 3], in_=pA[:])
                nc.tensor.matmul(out=pA[:], lhsT=W1T[:], rhs=A1d[:], start=True, stop=True)
                nc.vector.tensor_copy(out=W1ext[:, NH + 3:NH + 6], in_=pA[:])

                pT2 = sps.tile([NH, NH], BF16)
                nc.tensor.transpose(out=pT2[:], in_=W2sb[:], identity=ident[0:NH, 0:NH])
                W2T = cp.tile([NH, NH], BF16)
                nc.vector.tensor_copy(out=W2T[:], in_=pT2[:])
                W2ext = cp.tile([NH, 128], BF16)
                nc.vector.memset(W2ext[:], 0.0)
                nc.vector.tensor_copy(out=W2ext[:, 0:NH], in_=W2sb[:])
                pA2 = sps.tile([NH, cfg.H], F32)
                nc.tensor.matmul(out=pA2[:], lhsT=W2T[:], rhs=A2s[:], start=True, stop=True)
                nc.vector.tensor_copy(out=W2ext[:, NH:NH + 3], in_=pA2[:])
                nc.tensor.matmul(out=pA2[:], lhsT=W2T[:], rhs=A2d[:], start=True, stop=True)
                nc.vector.tensor_copy(out=W2ext[:, NH + 3:NH + 6], in_=pA2[:])

                # --- bias broadcast tiles (ones-matmul)
                ones = cp.tile([1, 128], F32)
                nc.vector.memset(ones[:], 1.0)
                b1r = cp.tile([128, NH], F32)
                b2r = cp.tile([128, cfg.F2], F32)
                fb1r = cp.tile([128, cfg.F2 // 2], F32)
                fb2r = cp.tile([128, 2], F32)
                for (rep, t_b, n) in ((b1r, t_b1, NH), (b2r, t_b2, cfg.F2),
                                      (fb1r, t_fb1, cfg.F2 // 2), (fb2r, t_fb2, 2)):
                    bsb = cp.tile([1, n], F32, tag="bias_stage")
                    nc.sync.dma_start(out=bsb[:], in_=t_b[None, :])
                    pb = sps.tile([128, n], F32, tag="bias_ps")
                    nc.tensor.matmul(out=pb[:], lhsT=ones[:], rhs=bsb[:], start=True, stop=True)
                    nc.vector.tensor_copy(out=rep[:], in_=pb[:])

            # --- pad rows
            pr = cp.tile([1, 128], BF16)
            nc.vector.memset(pr[:], 0.0)
            nc.vector.memset(pr[:, NH:NH + 3], -200.0)
            for tt, rr in ((table1, 0), (table1, T1R - 1), (table2, 0), (table2, T2R - 1)):
                nc.sync.dma_start(out=tt[rr:rr + 1, :], in_=pr[:])

            # ============================ dense 1 ============================
            if True:
              dense_phase(nc, tc, t_x, None, W1ext, table1, N, cfg.IN, idf,
                        cast_in=True)
              tc.strict_bb_all_engine_barrier()

            # ============================ layer 1 ============================
            if True:
              gat_layer(nc, tc, cfg, table1, t_idx1, SLO1, SHI1, LO_ROWS,
                      finalize="l1", b_rep=b1r, h_out=h1_shard,
                      consts=None)
              tc.strict_bb_all_engine_barrier()

            # ---------------------------- allgather --------------------------
            if True:
              nc.gpsimd.collective_compute(
                "AllGather", OP.bypass,
                replica_groups=[list(range(cfg.CORES))],
                ins=[h1_shard[:]], outs=[h1_all[:]])
              tc.strict_bb_all_engine_barrier()

            # ============================ dense 2 ============================
            if True:
              dense_phase(nc, tc, None, h1_all, W2ext, table2, NV, NH, ident,
                        cast_in=False)
              tc.strict_bb_all_engine_barrier()

            # ============================ layer 2 ============================
            if True:
              gat_layer(nc, tc, cfg, table2, t_idx2, SLO2, SHI2, LO_ROWS,
                      finalize="l2", b_rep=b2r, h_out=t_out,
                      consts=dict(fW1b=fW1b, fW2b=fW2b, fb1r=fb1r, fb2r=fb2r,
                                  ident=ident))

    nc.compile()
    return nc


def idx_cols(SLO, SHI):
    return int(8 * (SLO.sum() + SHI.sum()))


def dense_phase(nc, tc, t_x, h_all, Wext, table, NROWS, K, ident, cast_in):
    # ident dtype must match input dtype (f32 for cast_in, bf16 otherwise)
    """table[1+i] = [x[i] @ Wext]  (bf16, full 128-col rows).  4 tiles/group."""
    NCOLS = Wext.shape[1]
    GT = 4
    with (
        tc.tile_pool(name="dns", bufs=3) as dp,
        tc.tile_pool(name="dns_ps", bufs=2, space="PSUM") as pp,
    ):
        ntiles = (NROWS + 127) // 128
        for g0 in range(0, ntiles, GT):
            gn = min(GT, ntiles - g0)
            r0 = g0 * 128
            nr = min(GT * 128, NROWS - r0)
            full = nr == gn * 128
            in_dt = F32 if cast_in else BF16
            xt = dp.tile([128, GT * K], in_dt, tag="xt")
            x3 = xt[:].rearrange("p (t k) -> p t k", k=K)
            src_t = t_x if cast_in else h_all
            if full:
                nc.sync.dma_start(
                    out=x3[:, 0:gn, :],
                    in_=src_t[r0:r0 + nr, :].rearrange("(t p) k -> p t k", p=128))
            else:
                for t in range(gn):
                    nt = min(128, NROWS - (g0 + t) * 128)
                    nc.sync.dma_start(out=x3[:nt, t, :],
                                      in_=src_t[r0 + t * 128:r0 + t * 128 + nt, :])
            pt = pp.tile([K, GT * 128], in_dt, tag="pt")
            for t in range(gn):
                nt = min(128, NROWS - (g0 + t) * 128)
                nc.tensor.transpose(out=pt[:, t * 128:t * 128 + nt],
                                    in_=x3[:nt, t, :],
                                    identity=ident[:nt, :nt])
            xT = dp.tile([K, GT * 128], BF16, tag="xT")
            if full:
                nc.vector.tensor_copy(out=xT[:, 0:gn * 128], in_=pt[:, 0:gn * 128])
            else:
                for t in range(gn):
                    nt = min(128, NROWS - (g0 + t) * 128)
                    nc.vector.tensor_copy(out=xT[:, t * 128:t * 128 + nt],
                                          in_=pt[:, t * 128:t * 128 + nt])
            ph = pp.tile([128, GT * NCOLS], F32, tag="ph")
            for t in range(gn):
                nt = min(128, NROWS - (g0 + t) * 128)
                nc.tensor.matmul(out=ph[:nt, t * NCOLS:(t + 1) * NCOLS],
                                 lhsT=xT[:, t * 128:t * 128 + nt], rhs=Wext[:],
                                 start=True, stop=True)
            tb = dp.tile([128, GT * NCOLS], BF16, tag="tb")
            if full:
                nc.vector.tensor_copy(out=tb[:, 0:gn * NCOLS],
                                      in_=ph[:, 0:gn * NCOLS])
            else:
                for t in range(gn):
                    nt = min(128, NROWS - (g0 + t) * 128)
                    nc.vector.tensor_copy(
                        out=tb[:nt, t * NCOLS:(t + 1) * NCOLS],
                        in_=ph[:nt, t * NCOLS:(t + 1) * NCOLS])
            t3 = tb[:].rearrange("p (t k) -> p t k", k=NCOLS)
            if full:
                nc.sync.dma_start(
                    out=table[1 + r0:1 + r0 + nr, :].rearrange(
                        "(t p) k -> p t k", p=128),
                    in_=t3[:, 0:gn, :])
            else:
                for t in range(gn):
                    nt = min(128, NROWS - (g0 + t) * 128)
                    nc.sync.dma_start(
                        out=table[1 + r0 + t * 128:1 + r0 + t * 128 + nt, :],
                        in_=t3[:nt, t, :])


def gat_layer(nc, tc, cfg, table, t_idx, SLO, SHI, LO_ROWS, finalize,
              b_rep, h_out, consts):
    NH = cfg.NH1
    TR = table.shape[0]
    col_base = np.r_[0, np.cumsum(8 * (SLO + SHI))][:-1]
    with (
        tc.tile_pool(name="gat_g", bufs=3) as gp,
        tc.tile_pool(name="gat_w", bufs=3) as wp,
        tc.tile_pool(name="gat_ps", bufs=2, space="PSUM") as pp,
    ):
        for w in range(cfg.W):
            slo, shi = int(SLO[w]), int(SHI[w])
            S = slo + shi
            c0 = int(col_base[w])
            idxt = wp.tile([128, 8 * S], I16, tag="idx")
            nc.sync.dma_start(out=idxt[:], in_=t_idx[:, c0:c0 + 8 * S])
            g = gp.tile([128, S * 128], BF16, tag="g")
            g3 = g[:].rearrange("p (s f) -> p s f", f=128)
            GMAX = 8   # max 1024 idxs per dma_gather (desc-ring capacity)
            chunks = []
            for s0 in range(0, slo, GMAX):
                k = min(GMAX, slo - s0)
                chunks.append((s0, k, 0))
            for s0 in range(0, shi, GMAX):
                k = min(GMAX, shi - s0)
                chunks.append((slo + s0, k, 1))
            for ci, (s0, k, z) in enumerate(chunks):
                tab_ap = table[LO_ROWS:TR, :] if z else table[0:LO_ROWS, :]
                nc.gpsimd.dma_gather(
                    g3[:, s0:s0 + k, :], tab_ap,
                    idxt[:, 8 * s0:8 * (s0 + k)], k * 128, k * 128, 128,
                    queue_num=0)

            # a_d of own node: zone slot0 self rows (other zone slot0 = pad, a_d=0)
            adw = wp.tile([128, 3], F32, tag="adw")
            nc.vector.tensor_tensor(out=adw[:], in0=g3[:, 0, NH + 3:NH + 6],
                                    in1=g3[:, slo, NH + 3:NH + 6], op=OP.add)
            # e = a_s[src] + a_d[dst]  (head-major buffer [3, S])
            eb = wp.tile([128, 3 * S], F32, tag="eb")
            eb_sh = eb[:].rearrange("p (h s) -> p s h", s=S)
            g5 = g[:].rearrange("p (s h f) -> p s h f", h=4, f=32)
            nc.vector.tensor_tensor(out=eb_sh, in0=g5[:, :, 3, 0:3],
                                    in1=adw[:].unsqueeze(1).broadcast_to([128, S, 3]),
                                    op=OP.add)
            # leaky_relu(e) = e + 0.8*relu(-e)... e2 = relu(-e)*0.8 + e
            rb = wp.tile([128, 3 * S], F32, tag="rb")
            nc.scalar.activation(out=rb[:], in_=eb[:], func=AF.Relu, scale=-1.0)
            e2 = wp.tile([128, 3 * S], F32, tag="e2")
            nc.vector.affine_then_add(out=e2[:], in0=rb[:], in1=eb[:],
                                      scale=0.8, bias=0.0)
            # ex = exp(e2); denom[h] = sum_s ex
            exb = wp.tile([128, 3 * S], BF16, tag="exb")
            den = wp.tile([128, 3], F32, tag="den")
            for hd in range(3):
                nc.scalar.activation(out=exb[:, hd * S:(hd + 1) * S],
                                     in_=e2[:, hd * S:(hd + 1) * S],
                                     func=AF.Exp,
                                     accum_out=den[:, hd:hd + 1])
            # feat = h_src * ex  (bf16)
            feat = gp.tile([128, S * NH], BF16, tag="feat")
            f4 = feat[:].rearrange("p (s h f) -> p s h f", h=3, f=32)
            ex4 = (exb[:].rearrange("p (h s) -> p s h", s=S)
                   .unsqueeze(3).broadcast_to([128, S, 3, 32]))
            nc.vector.tensor_tensor(out=f4, in0=g5[:, :, 0:3, :], in1=ex4,
                                    op=OP.mult)
            # pairwise tree-reduce over slots -> num (f32)
            f3 = feat[:].rearrange("p (s f) -> p s f", f=NH)
            cur = S
            while cur > 2:
                hh = cur // 2
                if cur % 2:
                    nc.vector.tensor_tensor(out=f3[:, 0, :], in0=f3[:, 0, :],
                                            in1=f3[:, cur - 1, :], op=OP.add)
                nc.vector.tensor_tensor(out=f3[:, 0:hh, :], in0=f3[:, 0:hh, :],
                                        in1=f3[:, hh:2 * hh, :], op=OP.add)
                cur = hh
            num = wp.tile([128, NH], F32, tag="num")
            if cur == 2:
                nc.vector.tensor_tensor(out=num[:], in0=f3[:, 0, :],
                                        in1=f3[:, 1, :], op=OP.add)
            else:
                nc.vector.tensor_copy(out=num[:], in_=f3[:, 0, :])
            rec = wp.tile([128, 3], F32, tag="rec")
            nc.vector.reciprocal(rec[:], den[:])
            og = wp.tile([128, NH], F32, tag="og")
            nc.vector.tensor_tensor(
                out=og[:].rearrange("p (h f) -> p h f", f=32),
                in0=num[:].rearrange("p (h f) -> p h f", f=32),
                in1=rec[:].unsqueeze(2).broadcast_to([128, 3, 32]),
                op=OP.mult)

            if finalize == "l1":
                hb = wp.tile([128, NH], F32, tag="hb")
                nc.vector.tensor_tensor(out=hb[:], in0=og[:], in1=b_rep[:], op=OP.add)
                h1w = wp.tile([128, NH], BF16, tag="h1w")
                nc.scalar.activation(out=h1w[:], in_=hb[:], func=AF.Relu)
                nc.sync.dma_start(out=h_out[w * 128:(w + 1) * 128, :], in_=h1w[:])
            else:
                # mean over heads, + b2, relu
                hm = wp.tile([128, cfg.F2], F32, tag="hm")
                nc.vector.tensor_reduce(
                    out=hm[:], in_=og[:].rearrange("p (h f) -> p f h", f=32),
                    axis=AX.X, op=OP.add)
                hm2 = wp.tile([128, cfg.F2], F32, tag="hm2")
                nc.vector.affine_then_add(out=hm2[:], in0=hm[:], in1=b_rep[:],
                                          scale=1.0 / 3.0, bias=0.0)
                # note: b_rep added unscaled; hm scaled by 1/3
                h2t = wp.tile([128, cfg.F2], BF16, tag="h2t")
                nc.scalar.activation(out=h2t[:], in_=hm2[:], func=AF.Relu)
                # FF head
                ident = consts["ident"]
                pf = pp.tile([cfg.F2, 128], BF16, tag="pf")
                nc.tensor.transpose(out=pf[:], in_=h2t[:], identity=ident[:])
                t1 = wp.tile([cfg.F2, 128], BF16, tag="t1")
                nc.vector.tensor_copy(out=t1[:], in_=pf[:])
                pm1 = pp.tile([128, cfg.F2 // 2], F32, tag="pm1")
                nc.tensor.matmul(out=pm1[:], lhsT=t1[:], rhs=consts["fW1b"][:],
                                 start=True, stop=True)
                fo = wp.tile([128, cfg.F2 // 2], F32, tag="fo")
                nc.vector.tensor_tensor(out=fo[:], in0=pm1[:], in1=consts["fb1r"][:],
                                        op=OP.add)
                f1 = wp.tile([128, cfg.F2 // 2], BF16, tag="f1")
                nc.scalar.activation(out=f1[:], in_=fo[:], func=AF.Relu)
                pf2 = pp.tile([cfg.F2 // 2, 128], BF16, tag="pf2")
                nc.tensor.transpose(out=pf2[:], in_=f1[:], identity=ident[:])
                t2 = wp.tile([cfg.F2 // 2, 128], BF16, tag="t2")
                nc.vector.tensor_copy(out=t2[:], in_=pf2[:])
                pm2 = pp.tile([128, 2], F32, tag="pm2")
                nc.tensor.matmul(out=pm2[:], lhsT=t2[:], rhs=consts["fW2b"][:],
                                 start=True, stop=True)
                oo = wp.tile([128, 2], F32, tag="oo")
                nc.vector.tensor_tensor(out=oo[:], in0=pm2[:], in1=consts["fb2r"][:],
                                        op=OP.add)
                nc.sync.dma_start(out=h_out[w * 128:(w + 1) * 128, :], in_=oo[:])


# --------------------------------------------------------------------------
# entry points
# --------------------------------------------------------------------------

_INPUT_KEYS = ("x", "W1", "b1", "att_src1", "att_dst1", "W2", "b2",
               "att_src2", "att_dst2", "ffW1", "ffb1", "ffW2", "ffb2")


def _in_maps(plan, inputs):
    cfg = plan["cfg"]
    f32 = lambda a: np.ascontiguousarray(np.asarray(a, np.float32))
    base = {k: f32(inputs[k]) for k in _INPUT_KEYS}
    maps = []
    for c in range(cfg.CORES):
        m = dict(base)
        m["idx1"] = np.ascontiguousarray(plan["idx1"][c])
        m["idx2"] = np.ascontiguousarray(plan["idx2"][c])
        maps.append(m)
    return maps


def _assemble(plan, results):
    cfg = plan["cfg"]
    outs = np.stack([results[c]["out"] for c in range(cfg.CORES)])
    return outs[plan["core2"], plan["q2"]].astype(np.float32)


def kernel(**inputs):
    """Full-input GAT kernel on 8 NeuronCores; returns [50000, 2] float32."""
    cfg = Cfg()
    plan = make_plan(cfg, inputs["edge_index"])
    nc = build_program(plan)
    res = run_bass_kernel_spmd(nc, _in_maps(plan, inputs),
                               list(range(cfg.CORES)))
    return _assemble(plan, res.results)


def kernel_traced(**inputs):
    """kernel() + an HW-exec-time estimate in ns.

    No NTFF profiling is available in this container, so the estimate is a
    differential: amortized wall time of pipelined executions of the real
    NEFF minus the same measurement for a trivial NEFF taking identical
    inputs (isolating device execution from axon input-shipping overhead).
    """
    import time as _time
    import jax
    cfg = Cfg()
    plan = make_plan(cfg, inputs["edge_index"])
    nc = build_program(plan)
    maps = _in_maps(plan, inputs)
    res = run_bass_kernel_spmd(nc, maps, list(range(cfg.CORES)))
    out = _assemble(plan, res.results)

    exec_ns = None
    try:
        from concourse.bass2jax import (_bass_exec_p, install_neuronx_cc_hook,
                                        partition_id_tensor)
        from jax.sharding import Mesh, PartitionSpec
        from jax.experimental.shard_map import shard_map

        def make_exec(nc_):
            partition_name = (nc_.partition_id_tensor.name
                              if nc_.partition_id_tensor else None)
            in_names, out_names, out_avals, zero_outs = [], [], [], []
            for alloc in nc_.m.functions[0].allocations:
                if not isinstance(alloc, mybir.MemoryLocationSet):
                    continue
                name = alloc.memorylocations[0].name
                if alloc.kind == "ExternalInput":
                    if name != partition_name:
                        in_names.append(name)
                elif alloc.kind == "ExternalOutput":
                    shape = tuple(alloc.tensor_shape)
                    dtype = mybir.dt.np(alloc.dtype)
                    out_names.append(name)
                    out_avals.append(jax.core.ShapedArray(shape, dtype))
                    zero_outs.append(np.zeros(shape, dtype))

            def _body(*args):
                operands = list(args)
                if partition_name is not None:
                    operands.append(partition_id_tensor())
                return tuple(_bass_exec_p.bind(
                    *operands, out_avals=tuple(out_avals),
                    in_names=tuple(in_names + out_names +
                                   ([partition_name] if partition_name else [])),
                    out_names=tuple(out_names),
                    lowering_input_output_aliases=(),
                    sim_require_finite=True, sim_require_nnan=True, nc=nc_))

            devices = jax.devices()[:cfg.CORES]
            mesh = Mesh(np.asarray(devices), ("core",))
            nin = len(in_names) + len(zero_outs)
            fn = jax.jit(shard_map(_body, mesh=mesh,
                                   in_specs=(PartitionSpec("core"),) * nin,
                                   out_specs=(PartitionSpec("core"),) * len(out_names),
                                   check_rep=False), keep_unused=True)
            cat = [np.concatenate([np.asarray(maps[c][nm])
                                   for c in range(cfg.CORES)], axis=0)
                   for nm in in_names]
            cat += [np.zeros((cfg.CORES * z.shape[0], *z.shape[1:]), z.dtype)
                    for z in zero_outs]
            return fn, cat

        def amortized(fn, args, iters=20):
            o = fn(*args); jax.block_until_ready(o)
            t0 = _time.perf_counter()
            os_ = [fn(*args) for _ in range(iters)]
            jax.block_until_ready(os_)
            return (_time.perf_counter() - t0) / iters

        install_neuronx_cc_hook()
        fn, args = make_exec(nc)
        t_kernel = amortized(fn, args)

        nc2 = _floor_program(plan)
        fn2, args2 = make_exec(nc2)
        t_floor = amortized(fn2, args2)
        exec_ns = max(0.0, (t_kernel - t_floor)) * 1e9
    except Exception as e:
        print("timing estimate failed:", str(e)[:200])
    return out, (int(exec_ns) if exec_ns is not None else None)


def _floor_program(plan):
    """Trivial NEFF with identical I/O signature, for differential timing."""
    cfg = plan["cfg"]
    C1 = idx_cols(plan["SLO1"], plan["SHI1"])
    C2 = idx_cols(plan["SLO2"], plan["SHI2"])
    nc = Bacc("TRN2", target_bir_lowering=False, debug=False,
              num_devices=cfg.CORES)
    shapes = dict(x=[cfg.N, cfg.IN], W1=[cfg.IN, 96], b1=[96],
                  att_src1=[3, 32], att_dst1=[3, 32], W2=[96, 96], b2=[32],
                  att_src2=[3, 32], att_dst2=[3, 32], ffW1=[32, 16],
                  ffb1=[16], ffW2=[16, 2], ffb2=[2])
    for k, s in shapes.items():
        nc.dram_tensor(k, s, F32, kind="ExternalInput")
    nc.dram_tensor("idx1", [128, C1], I16, kind="ExternalInput")
    nc.dram_tensor("idx2", [128, C2], I16, kind="ExternalInput")
    t_o = nc.dram_tensor("out", [cfg.NPC, 2], F32, kind="ExternalOutput")
    with tile.TileContext(nc, num_cores=cfg.CORES) as tc:
        with tc.tile_pool(name="p", bufs=1) as p:
            ta = p.tile([128, 2], F32)
            nc.vector.memset(ta[:], 1.0)
            nc.sync.dma_start(out=t_o[0:128, :], in_=ta[:])
    nc.compile()
    return nc



# revision 26
# speedup vs baseline: 69.2564x; 69.2564x over previous
"""Two-layer GAT on 8 TRN2 NeuronCores.

Strategy (edge-parallel, dst-sharded):
  - Host planner partitions dst nodes across 8 cores into degree-sorted
    windows of 128 nodes (lanes).  Each node's incoming edges occupy
    "slots" along the SBUF free dim; every window is padded to its max
    per-zone degree.  Per-edge tables rows (h | a_s | a_d) are fetched
    with dma_gather keyed by src.  Softmax (without max-subtraction --
    logits are bounded) and the weighted sum are computed per-lane with
    free-dim reductions; no scatter is ever needed on device.
  - int16 gather indices cap the index range at 32767, so each table is
    addressed in two zones (lo/hi rows) with per-node slot-zones.
  - Between layers one AllGather replicates the produced h1 rows.

Layout of a table row (128 bf16 = 256B):
  [0:96] h (post-linear, pre-attention)  [96:99] a_s  [99:102] a_d
Row 0 and the last row are pad rows: h=0, a_s=-200 (=> exp ~ 0), a_d=0.
"""

import dataclasses
import numpy as np

from concourse import bass, mybir, tile
from concourse.bacc import Bacc
from concourse.bass_utils import run_bass_kernel_spmd
from concourse.masks import make_identity

F32 = mybir.dt.float32
BF16 = mybir.dt.bfloat16
I16 = mybir.dt.int16
AX = mybir.AxisListType
OP = mybir.AluOpType
AF = mybir.ActivationFunctionType


@dataclasses.dataclass
class Cfg:
    N: int = 50000          # real nodes
    H: int = 3
    IN: int = 128
    F1: int = 32            # per-head feats layer1 (concat -> 96)
    F2: int = 32
    CORES: int = 8
    LANES: int = 128
    THR: int = 32766        # node/pos <= THR -> lo zone (idx = v+1 <= 32767)

    @property
    def NH1(self):
        return self.H * self.F1    # 96

    @property
    def NV(self):
        per = self.CORES * self.LANES
        return ((self.N + per - 1) // per) * per

    @property
    def W(self):
        return self.NV // (self.CORES * self.LANES)

    @property
    def NPC(self):
        return self.W * self.LANES


def _layout_for_layer(cfg, dst_s, srckey_s, selfmask_s):
    """Compute slots/zones for one layer.

    dst_s:     [E] dst node ids, sorted ascending (stable)
    srckey_s:  [E] gather key of the src endpoint (node id or perm pos)
    selfmask_s:[E] bool, true iff edge is a self loop (src==dst)
    Returns dict(slot, zone, lo_need, hi_need) -- slot/zone per edge.
    """
    N = cfg.N
    E = len(dst_s)
    zone = (srckey_s > cfg.THR).astype(np.int8)

    # one designated self edge per node -> slot 0 of its zone
    selfpos = np.full(N, -1, np.int64)
    cand = np.nonzero(selfmask_s)[0][::-1]
    selfpos[dst_s[cand]] = cand          # first occurrence wins
    assert (selfpos >= 0).all(), "every node needs a self loop"
    designated = np.zeros(E, bool)
    designated[selfpos] = True

    # ranks within (node, zone) among non-designated edges
    norm_idx = np.nonzero(~designated)[0]
    key = dst_s[norm_idx].astype(np.int64) * 2 + zone[norm_idx]
    order = np.argsort(key, kind="stable")
    ksort = key[order]
    grp_start = np.r_[0, np.nonzero(np.diff(ksort))[0] + 1]
    start_of = np.repeat(grp_start, np.diff(np.r_[grp_start, len(ksort)]))
    rank_sorted = np.arange(len(ksort)) - start_of
    slot = np.empty(E, np.int32)
    slot[norm_idx[order]] = rank_sorted + 1        # slots 1..
    slot[designated] = 0

    nlo_e = np.bincount(dst_s[zone == 0], minlength=N)   # lo edges incl self
    nhi_e = np.bincount(dst_s[zone == 1], minlength=N)
    node_hi = np.zeros(N, bool)
    node_hi[dst_s[designated]] = zone[designated] == 1
    lo_need = nlo_e + node_hi.astype(np.int64)           # +1 pad slot 0 if self is hi
    hi_need = nhi_e + (~node_hi).astype(np.int64)
    return dict(slot=slot, zone=zone, lo_need=lo_need, hi_need=hi_need)


def _windows(cfg, lo_need, hi_need):
    """Sort nodes lex by (lo_need, hi_need); deal to cores; window maxima."""
    N, NV, W = cfg.N, cfg.NV, cfg.W
    order = np.lexsort((hi_need, lo_need))       # [N] node ids by rank
    rank_of = np.empty(N, np.int64)
    rank_of[order] = np.arange(N)
    # rank r -> core r%C, q r//C; window q//LANES, lane q%LANES
    lo_v = np.zeros(NV, np.int64)
    hi_v = np.zeros(NV, np.int64)
    lo_v[: N] = lo_need[order]
    hi_v[: N] = hi_need[order]
    per_w = cfg.CORES * cfg.LANES
    SLO = np.maximum(1, lo_v.reshape(W, per_w).max(1)).astype(np.int64)
    SHI = np.maximum(1, hi_v.reshape(W, per_w).max(1)).astype(np.int64)
    return order, rank_of, SLO, SHI


def _fill_idx(cfg, rank_of, SLO, SHI, dst_s, slot, zone, idxval, pad_hi_idx):
    """Build per-core wrapped idx arrays [128, C] int16.

    Column layout: for w in range(W): [8*SLO[w] lo cols][8*SHI[w] hi cols].
    """
    W, LANES, CORES = cfg.W, cfg.LANES, cfg.CORES
    colw = 8 * (SLO + SHI)
    col_base = np.r_[0, np.cumsum(colw)][:-1]
    C = int(colw.sum())
    arrs = np.zeros((CORES, 16, C), np.int16)
    # prefill hi zones with their pad idx (lo pad idx is 0 already)
    for w in range(W):
        h0 = col_base[w] + 8 * SLO[w]
        arrs[:, :, h0: h0 + 8 * SHI[w]] = pad_hi_idx

    rank = rank_of[dst_s]
    core = rank % CORES
    q = rank // CORES
    lane = q % LANES
    wno = q // LANES
    base = col_base[wno] + np.where(zone == 1, 8 * SLO[wno], 0)
    i = slot.astype(np.int64) * LANES + lane
    col = base + i // 16
    row = i % 16
    v = idxval.astype(np.int16)
    arrs[core, row, col] = v
    out = np.zeros((CORES, 128, C), np.int16)
    for k in range(8):
        out[:, k * 16:(k + 1) * 16, :] = arrs
    return out, col_base


def make_plan(cfg, edge_index):
    N = cfg.N
    ei = np.asarray(edge_index)
    src = np.concatenate([ei[0], np.arange(N, dtype=np.int64)]).astype(np.int64)
    dst = np.concatenate([ei[1], np.arange(N, dtype=np.int64)]).astype(np.int64)
    eo = np.argsort(dst, kind="stable")
    src_s, dst_s = src[eo], dst[eo]
    selfmask_s = src_s == dst_s
    deg = np.bincount(dst_s, minlength=N)

    # ---- layer 1 (gather key = original node id)
    l1 = _layout_for_layer(cfg, dst_s, src_s, selfmask_s)
    order1, rank1, SLO1, SHI1 = _windows(cfg, l1["lo_need"], l1["hi_need"])
    idxval1 = np.where(l1["zone"] == 0, src_s + 1, src_s - (cfg.THR + 1))
    pad_hi1 = cfg.N - cfg.THR - 1             # row N+1 is pad2; hi base row THR+2
    idx1, _ = _fill_idx(cfg, rank1, SLO1, SHI1, dst_s, l1["slot"], l1["zone"],
                        idxval1, pad_hi1)

    # pos1: node -> row in h1_all  (core-major concat of per-core window rows)
    core1 = rank1 % cfg.CORES
    q1 = rank1 // cfg.CORES
    pos1 = core1 * cfg.NPC + q1               # [N]

    # ---- layer 2 (gather key = pos1 of src)
    key2_s = pos1[src_s]
    l2 = _layout_for_layer(cfg, dst_s, key2_s, selfmask_s)
    order2, rank2, SLO2, SHI2 = _windows(cfg, l2["lo_need"], l2["hi_need"])
    idxval2 = np.where(l2["zone"] == 0, key2_s + 1, key2_s - (cfg.THR + 1))
    pad_hi2 = cfg.NV - cfg.THR - 1
    idx2, _ = _fill_idx(cfg, rank2, SLO2, SHI2, dst_s, l2["slot"], l2["zone"],
                        idxval2, pad_hi2)

    core2 = rank2 % cfg.CORES
    q2 = rank2 // cfg.CORES
    return dict(cfg=cfg, SLO1=SLO1, SHI1=SHI1, SLO2=SLO2, SHI2=SHI2,
                idx1=idx1, idx2=idx2, core2=core2, q2=q2)


# --------------------------------------------------------------------------
# device program
# --------------------------------------------------------------------------

def build_program(plan, phases=("dense1", "layer1", "ag", "dense2", "layer2")):
    phases = set(phases)
    cfg: Cfg = plan["cfg"]
    N, NV, W, NPC = cfg.N, cfg.NV, cfg.W, cfg.NPC
    NH = cfg.NH1                      # 96
    SLO1, SHI1 = plan["SLO1"], plan["SHI1"]
    SLO2, SHI2 = plan["SLO2"], plan["SHI2"]
    C1 = idx_cols(SLO1, SHI1)
    C2 = idx_cols(SLO2, SHI2)
    T1R = N + 2                        # table1 rows (pad + N + pad)
    T2R = NV + 2
    LO_ROWS = cfg.THR + 2              # rows [0, THR+2): pad + keys 0..THR
    nc = Bacc("TRN2", target_bir_lowering=False, debug=False,
              num_devices=cfg.CORES, num_swdge_queues=4)

    t_x = nc.dram_tensor("x", [N, cfg.IN], F32, kind="ExternalInput")
    t_W1 = nc.dram_tensor("W1", [cfg.IN, NH], F32, kind="ExternalInput")
    t_b1 = nc.dram_tensor("b1", [NH], F32, kind="ExternalInput")
    t_as1 = nc.dram_tensor("att_src1", [cfg.H, cfg.F1], F32, kind="ExternalInput")
    t_ad1 = nc.dram_tensor("att_dst1", [cfg.H, cfg.F1], F32, kind="ExternalInput")
    t_W2 = nc.dram_tensor("W2", [NH, NH], F32, kind="ExternalInput")
    t_b2 = nc.dram_tensor("b2", [cfg.F2], F32, kind="ExternalInput")
    t_as2 = nc.dram_tensor("att_src2", [cfg.H, cfg.F2], F32, kind="ExternalInput")
    t_ad2 = nc.dram_tensor("att_dst2", [cfg.H, cfg.F2], F32, kind="ExternalInput")
    t_fW1 = nc.dram_tensor("ffW1", [cfg.F2, cfg.F2 // 2], F32, kind="ExternalInput")
    t_fb1 = nc.dram_tensor("ffb1", [cfg.F2 // 2], F32, kind="ExternalInput")
    t_fW2 = nc.dram_tensor("ffW2", [cfg.F2 // 2, 2], F32, kind="ExternalInput")
    t_fb2 = nc.dram_tensor("ffb2", [2], F32, kind="ExternalInput")
    t_idx1 = nc.dram_tensor("idx1", [128, C1], I16, kind="ExternalInput")
    t_idx2 = nc.dram_tensor("idx2", [128, C2], I16, kind="ExternalInput")
    t_out = nc.dram_tensor("out", [NPC, 2], F32, kind="ExternalOutput")

    table1 = nc.dram_tensor("table1", [T1R, 128], BF16)
    table2 = nc.dram_tensor("table2", [T2R, 128], BF16)
    h1_shard = nc.dram_tensor("h1_shard", [NPC, NH], BF16)
    h1_all = nc.dram_tensor("h1_all", [NV, NH], BF16, addr_space="Shared")

    with tile.TileContext(nc, num_cores=cfg.CORES) as tc:
        with tc.tile_pool(name="const", bufs=1) as cp:
            idf = cp.tile([128, 128], F32)
            make_identity(nc, idf[:])
            ident = cp.tile([128, 128], BF16)
            nc.vector.tensor_copy(out=ident[:], in_=idf[:])

            # --- weights
            W1sb = cp.tile([128, NH], BF16)
            nc.gpsimd.dma_start(out=W1sb[:], in_=t_W1[:])
            W2sb = cp.tile([NH, NH], BF16)
            nc.gpsimd.dma_start(out=W2sb[:], in_=t_W2[:])
            fW1b = cp.tile([cfg.F2, cfg.F2 // 2], BF16)
            nc.gpsimd.dma_start(out=fW1b[:], in_=t_fW1[:])
            fW2b = cp.tile([cfg.F2 // 2, 2], BF16)
            nc.gpsimd.dma_start(out=fW2b[:], in_=t_fW2[:])

            A1s = cp.tile([NH, cfg.H], BF16)
            A1d = cp.tile([NH, cfg.H], BF16)
            A2s = cp.tile([NH, cfg.H], BF16)
            A2d = cp.tile([NH, cfg.H], BF16)
            for (a_t, t_a, Ff) in ((A1s, t_as1, cfg.F1), (A1d, t_ad1, cfg.F1),
                                   (A2s, t_as2, cfg.F2), (A2d, t_ad2, cfg.F2)):
                nc.vector.memset(a_t[:], 0.0)
                for hd in range(cfg.H):
                    nc.gpsimd.dma_start(
                        out=a_t[hd * Ff:(hd + 1) * Ff, hd:hd + 1],
                        in_=t_a[hd, :, None])

            with tc.tile_pool(name="setup_ps", bufs=1, space="PSUM") as sps:
                # W1ext = [W1 | W1@A1s | W1@A1d]
                pT = sps.tile([128, 128], BF16)
                nc.tensor.transpose(out=pT[0:NH, :], in_=W1sb[:], identity=ident[:])
                W1T = cp.tile([NH, 128], BF16)
                nc.vector.tensor_copy(out=W1T[:], in_=pT[0:NH, :])
                W1ext = cp.tile([128, 128], BF16)
                nc.vector.memset(W1ext[:], 0.0)
                nc.vector.tensor_copy(out=W1ext[:, 0:NH], in_=W1sb[:])
                pA = sps.tile([128, cfg.H], F32)
                nc.tensor.matmul(out=pA[:], lhsT=W1T[:], rhs=A1s[:], start=True, stop=True)
                nc.vector.tensor_copy(out=W1ext[:, NH:NH + 3], in_=pA[:])
                nc.tensor.matmul(out=pA[:], lhsT=W1T[:], rhs=A1d[:], start=True, stop=True)
                nc.vector.tensor_copy(out=W1ext[:, NH + 3:NH + 6], in_=pA[:])

                pT2 = sps.tile([NH, NH], BF16)
                nc.tensor.transpose(out=pT2[:], in_=W2sb[:], identity=ident[0:NH, 0:NH])
                W2T = cp.tile([NH, NH], BF16)
                nc.vector.tensor_copy(out=W2T[:], in_=pT2[:])
                W2ext = cp.tile([NH, 128], BF16)
                nc.vector.memset(W2ext[:], 0.0)
                nc.vector.tensor_copy(out=W2ext[:, 0:NH], in_=W2sb[:])
                pA2 = sps.tile([NH, cfg.H], F32)
                nc.tensor.matmul(out=pA2[:], lhsT=W2T[:], rhs=A2s[:], start=True, stop=True)
                nc.vector.tensor_copy(out=W2ext[:, NH:NH + 3], in_=pA2[:])
                nc.tensor.matmul(out=pA2[:], lhsT=W2T[:], rhs=A2d[:], start=True, stop=True)
                nc.vector.tensor_copy(out=W2ext[:, NH + 3:NH + 6], in_=pA2[:])

                # --- bias broadcast tiles (ones-matmul)
                ones = cp.tile([1, 128], F32)
                nc.vector.memset(ones[:], 1.0)
                b1r = cp.tile([128, NH], F32)
                b2r = cp.tile([128, cfg.F2], F32)
                fb1r = cp.tile([128, cfg.F2 // 2], F32)
                fb2r = cp.tile([128, 2], F32)
                for (rep, t_b, n) in ((b1r, t_b1, NH), (b2r, t_b2, cfg.F2),
                                      (fb1r, t_fb1, cfg.F2 // 2), (fb2r, t_fb2, 2)):
                    bsb = cp.tile([1, n], F32, tag="bias_stage")
                    nc.sync.dma_start(out=bsb[:], in_=t_b[None, :])
                    pb = sps.tile([128, n], F32, tag="bias_ps")
                    nc.tensor.matmul(out=pb[:], lhsT=ones[:], rhs=bsb[:], start=True, stop=True)
                    nc.vector.tensor_copy(out=rep[:], in_=pb[:])

            # --- pad rows
            pr = cp.tile([1, 128], BF16)
            nc.vector.memset(pr[:], 0.0)
            nc.vector.memset(pr[:, NH:NH + 3], -200.0)
            for tt, rr in ((table1, 0), (table1, T1R - 1), (table2, 0), (table2, T2R - 1)):
                nc.sync.dma_start(out=tt[rr:rr + 1, :], in_=pr[:])

            # ============================ dense 1 ============================
            if "dense1" in phases:
              dense_phase(nc, tc, t_x, None, W1ext, table1, N, cfg.IN, idf,
                        cast_in=True)
              tc.strict_bb_all_engine_barrier()

            # ============================ layer 1 ============================
            l1p = [p for p in phases if p.startswith("layer1")]
            if l1p:
              sa1 = {"layer1gath": "gath"}.get(l1p[0])
              gat_layer(nc, tc, cfg, table1, t_idx1, SLO1, SHI1, LO_ROWS,
                      finalize="l1", b_rep=b1r, h_out=h1_shard,
                      consts=None, stop_after=sa1)
              tc.strict_bb_all_engine_barrier()

            # -------- probe: second copy of layer1 gathers ------------------
            if "layer1gathB" in phases:
              gat_layer(nc, tc, cfg, table1, t_idx1, SLO1, SHI1, LO_ROWS,
                      finalize="l1", b_rep=b1r, h_out=h1_shard,
                      consts=None, stop_after="gath")
              tc.strict_bb_all_engine_barrier()

            # ---------------------------- allgather --------------------------
            if "ag" in phases:
              nc.gpsimd.collective_compute(
                "AllGather", OP.bypass,
                replica_groups=[list(range(cfg.CORES))],
                ins=[h1_shard[:]], outs=[h1_all[:]])
              tc.strict_bb_all_engine_barrier()

            # -------- probe: layer1 gathers after the collective ------------
            if "layer1gathC" in phases:
              gat_layer(nc, tc, cfg, table1, t_idx1, SLO1, SHI1, LO_ROWS,
                      finalize="l1", b_rep=b1r, h_out=h1_shard,
                      consts=None, stop_after="gath")
              tc.strict_bb_all_engine_barrier()

            # ============================ dense 2 ============================
            if "dense2" in phases:
              dense_phase(nc, tc, None, h1_all, W2ext, table2, NV, NH, ident,
                        cast_in=False)
              tc.strict_bb_all_engine_barrier()

            # ============================ layer 2 ============================
            l2p = [p for p in phases if p.startswith("layer2")]
            if l2p:
              sa = {"layer2gath": "gath", "layer2att": "att",
                    "layer2feat": "feat", "layer2swap": "gath"}.get(l2p[0])
              if l2p[0] == "layer2swap":
                  # timing probe: layer2 position, but layer1's idx/slots
                  gat_layer(nc, tc, cfg, table2, t_idx1, SLO1, SHI1, LO_ROWS,
                          finalize="l2", b_rep=b2r, h_out=t_out,
                          consts=None, stop_after="gath")
              else:
                  gat_layer(nc, tc, cfg, table2, t_idx2, SLO2, SHI2, LO_ROWS,
                          finalize="l2", b_rep=b2r, h_out=t_out,
                          consts=dict(fW1b=fW1b, fW2b=fW2b, fb1r=fb1r,
                                      fb2r=fb2r, ident=ident),
                          skip_ff=l2p[0] == "layer2noff", stop_after=sa)

    nc.compile()
    return nc


def idx_cols(SLO, SHI):
    return int(8 * (SLO.sum() + SHI.sum()))


def dense_phase(nc, tc, t_x, h_all, Wext, table, NROWS, K, ident, cast_in):
    # ident dtype must match input dtype (f32 for cast_in, bf16 otherwise)
    """table[1+i] = [x[i] @ Wext]  (bf16, full 128-col rows).  4 tiles/group."""
    NCOLS = Wext.shape[1]
    GT = 4
    with (
        tc.tile_pool(name="dns", bufs=3) as dp,
        tc.tile_pool(name="dns_ps", bufs=2, space="PSUM") as pp,
    ):
        ntiles = (NROWS + 127) // 128
        for g0 in range(0, ntiles, GT):
            gn = min(GT, ntiles - g0)
            r0 = g0 * 128
            nr = min(GT * 128, NROWS - r0)
            full = nr == gn * 128
            in_dt = F32 if cast_in else BF16
            xt = dp.tile([128, GT * K], in_dt, tag="xt")
            x3 = xt[:].rearrange("p (t k) -> p t k", k=K)
            src_t = t_x if cast_in else h_all
            if full:
                nc.sync.dma_start(
                    out=x3[:, 0:gn, :],
                    in_=src_t[r0:r0 + nr, :].rearrange("(t p) k -> p t k", p=128))
            else:
                for t in range(gn):
                    nt = min(128, NROWS - (g0 + t) * 128)
                    nc.sync.dma_start(out=x3[:nt, t, :],
                                      in_=src_t[r0 + t * 128:r0 + t * 128 + nt, :])
            pt = pp.tile([K, GT * 128], in_dt, tag="pt")
            for t in range(gn):
                nt = min(128, NROWS - (g0 + t) * 128)
                nc.tensor.transpose(out=pt[:, t * 128:t * 128 + nt],
                                    in_=x3[:nt, t, :],
                                    identity=ident[:nt, :nt])
            xT = dp.tile([K, GT * 128], BF16, tag="xT")
            if full:
                nc.vector.tensor_copy(out=xT[:, 0:gn * 128], in_=pt[:, 0:gn * 128])
            else:
                for t in range(gn):
                    nt = min(128, NROWS - (g0 + t) * 128)
                    nc.vector.tensor_copy(out=xT[:, t * 128:t * 128 + nt],
                                          in_=pt[:, t * 128:t * 128 + nt])
            ph = pp.tile([128, GT * NCOLS], F32, tag="ph")
            for t in range(gn):
                nt = min(128, NROWS - (g0 + t) * 128)
                nc.tensor.matmul(out=ph[:nt, t * NCOLS:(t + 1) * NCOLS],
                                 lhsT=xT[:, t * 128:t * 128 + nt], rhs=Wext[:],
                                 start=True, stop=True)
            tb = dp.tile([128, GT * NCOLS], BF16, tag="tb")
            if full:
                nc.vector.tensor_copy(out=tb[:, 0:gn * NCOLS],
                                      in_=ph[:, 0:gn * NCOLS])
            else:
                for t in range(gn):
                    nt = min(128, NROWS - (g0 + t) * 128)
                    nc.vector.tensor_copy(
                        out=tb[:nt, t * NCOLS:(t + 1) * NCOLS],
                        in_=ph[:nt, t * NCOLS:(t + 1) * NCOLS])
            t3 = tb[:].rearrange("p (t k) -> p t k", k=NCOLS)
            if full:
                nc.sync.dma_start(
                    out=table[1 + r0:1 + r0 + nr, :].rearrange(
                        "(t p) k -> p t k", p=128),
                    in_=t3[:, 0:gn, :])
            else:
                for t in range(gn):
                    nt = min(128, NROWS - (g0 + t) * 128)
                    nc.sync.dma_start(
                        out=table[1 + r0 + t * 128:1 + r0 + t * 128 + nt, :],
                        in_=t3[:nt, t, :])


def gat_layer(nc, tc, cfg, table, t_idx, SLO, SHI, LO_ROWS, finalize,
              b_rep, h_out, consts, skip_ff=False, stop_after=None):
    NH = cfg.NH1
    TR = table.shape[0]
    col_base = np.r_[0, np.cumsum(8 * (SLO + SHI))][:-1]
    with (
        tc.tile_pool(name="gat_g", bufs=3) as gp,
        tc.tile_pool(name="gat_w", bufs=3) as wp,
        tc.tile_pool(name="gat_ps", bufs=2, space="PSUM") as pp,
    ):
        gq = 0
        for w in range(cfg.W):
            slo, shi = int(SLO[w]), int(SHI[w])
            S = slo + shi
            c0 = int(col_base[w])
            idxt = wp.tile([128, 8 * S], I16, tag="idx")
            nc.sync.dma_start(out=idxt[:], in_=t_idx[:, c0:c0 + 8 * S])
            g = gp.tile([128, S * 128], BF16, tag="g")
            g3 = g[:].rearrange("p (s f) -> p s f", f=128)
            GMAX = 8   # max 1024 idxs per dma_gather (desc-ring capacity;
                       # 2048 with a doubled scratch hangs the ucode)
            chunks = []
            for s0 in range(0, slo, GMAX):
                k = min(GMAX, slo - s0)
                chunks.append((s0, k, 0))
            for s0 in range(0, shi, GMAX):
                k = min(GMAX, shi - s0)
                chunks.append((slo + s0, k, 1))
            for ci, (s0, k, z) in enumerate(chunks):
                tab_ap = table[LO_ROWS:TR, :] if z else table[0:LO_ROWS, :]
                nc.gpsimd.dma_gather(
                    g3[:, s0:s0 + k, :], tab_ap,
                    idxt[:, 8 * s0:8 * (s0 + k)], k * 128, k * 128, 128,
                    queue_num=gq % 4)
                gq += 1

            odt = F32 if finalize == "l2" else BF16
            if stop_after == "gath":
                dmy = wp.tile([128, 2], odt, tag="dmy")
                nc.vector.tensor_copy(out=dmy[:], in_=g3[:, 0, 0:2])
                nc.sync.dma_start(out=h_out[w * 128:(w + 1) * 128, 0:2],
                                  in_=dmy[:])
                continue

            # a_d of own node: zone slot0 self rows (other zone slot0 = pad, a_d=0)
            adw = wp.tile([128, 3], F32, tag="adw")
            nc.vector.tensor_tensor(out=adw[:], in0=g3[:, 0, NH + 3:NH + 6],
                                    in1=g3[:, slo, NH + 3:NH + 6], op=OP.add)
            # e = a_s[src] + a_d[dst]  (head-major buffer [3, S])
            eb = wp.tile([128, 3 * S], F32, tag="eb")
            eb_sh = eb[:].rearrange("p (h s) -> p s h", s=S)
            g5 = g[:].rearrange("p (s h f) -> p s h f", h=4, f=32)
            nc.vector.tensor_tensor(out=eb_sh, in0=g5[:, :, 3, 0:3],
                                    in1=adw[:].unsqueeze(1).broadcast_to([128, S, 3]),
                                    op=OP.add)
            # leaky_relu(e) = e + 0.8*relu(-e)... e2 = relu(-e)*0.8 + e
            rb = wp.tile([128, 3 * S], F32, tag="rb")
            nc.scalar.activation(out=rb[:], in_=eb[:], func=AF.Relu, scale=-1.0)
            e2 = wp.tile([128, 3 * S], F32, tag="e2")
            nc.vector.affine_then_add(out=e2[:], in0=rb[:], in1=eb[:],
                                      scale=0.8, bias=0.0)
            # ex = exp(e2); denom[h] = sum_s ex
            exb = wp.tile([128, 3 * S], BF16, tag="exb")
            den = wp.tile([128, 3], F32, tag="den")
            for hd in range(3):
                nc.scalar.activation(out=exb[:, hd * S:(hd + 1) * S],
                                     in_=e2[:, hd * S:(hd + 1) * S],
                                     func=AF.Exp,
                                     accum_out=den[:, hd:hd + 1])
            if stop_after == "att":
                dmy = wp.tile([128, 2], odt, tag="dmy")
                nc.vector.tensor_copy(out=dmy[:], in_=den[:, 0:2])
                nc.sync.dma_start(out=h_out[w * 128:(w + 1) * 128, 0:2],
                                  in_=dmy[:])
                continue
            # feat = h_src * ex  (bf16)
            feat = gp.tile([128, S * NH], BF16, tag="feat")
            f4 = feat[:].rearrange("p (s h f) -> p s h f", h=3, f=32)
            ex4 = (exb[:].rearrange("p (h s) -> p s h", s=S)
                   .unsqueeze(3).broadcast_to([128, S, 3, 32]))
            nc.vector.tensor_tensor(out=f4, in0=g5[:, :, 0:3, :], in1=ex4,
                                    op=OP.mult)
            # pairwise tree-reduce over slots -> num (f32)
            f3 = feat[:].rearrange("p (s f) -> p s f", f=NH)
            cur = S
            while cur > 2:
                hh = cur // 2
                if cur % 2:
                    nc.vector.tensor_tensor(out=f3[:, 0, :], in0=f3[:, 0, :],
                                            in1=f3[:, cur - 1, :], op=OP.add)
                nc.vector.tensor_tensor(out=f3[:, 0:hh, :], in0=f3[:, 0:hh, :],
                                        in1=f3[:, hh:2 * hh, :], op=OP.add)
                cur = hh
            num = wp.tile([128, NH], F32, tag="num")
            if cur == 2:
                nc.vector.tensor_tensor(out=num[:], in0=f3[:, 0, :],
                                        in1=f3[:, 1, :], op=OP.add)
            else:
                nc.vector.tensor_copy(out=num[:], in_=f3[:, 0, :])
            if stop_after == "feat":
                dmy = wp.tile([128, 2], odt, tag="dmy")
                nc.vector.tensor_copy(out=dmy[:], in_=num[:, 0:2])
                nc.sync.dma_start(out=h_out[w * 128:(w + 1) * 128, 0:2],
                                  in_=dmy[:])
                continue
            rec = wp.tile([128, 3], F32, tag="rec")
            nc.vector.reciprocal(rec[:], den[:])
            og = wp.tile([128, NH], F32, tag="og")
            nc.vector.tensor_tensor(
                out=og[:].rearrange("p (h f) -> p h f", f=32),
                in0=num[:].rearrange("p (h f) -> p h f", f=32),
                in1=rec[:].unsqueeze(2).broadcast_to([128, 3, 32]),
                op=OP.mult)

            if finalize == "l1":
                hb = wp.tile([128, NH], F32, tag="hb")
                nc.vector.tensor_tensor(out=hb[:], in0=og[:], in1=b_rep[:], op=OP.add)
                h1w = wp.tile([128, NH], BF16, tag="h1w")
                nc.scalar.activation(out=h1w[:], in_=hb[:], func=AF.Relu)
                nc.sync.dma_start(out=h_out[w * 128:(w + 1) * 128, :], in_=h1w[:])
            else:
                # mean over heads, + b2, relu
                hm = wp.tile([128, cfg.F2], F32, tag="hm")
                nc.vector.tensor_reduce(
                    out=hm[:], in_=og[:].rearrange("p (h f) -> p f h", f=32),
                    axis=AX.X, op=OP.add)
                hm2 = wp.tile([128, cfg.F2], F32, tag="hm2")
                nc.vector.affine_then_add(out=hm2[:], in0=hm[:], in1=b_rep[:],
                                          scale=1.0 / 3.0, bias=0.0)
                # note: b_rep added unscaled; hm scaled by 1/3
                h2t = wp.tile([128, cfg.F2], BF16, tag="h2t")
                nc.scalar.activation(out=h2t[:], in_=hm2[:], func=AF.Relu)
                if skip_ff:
                    oo = wp.tile([128, 2], F32, tag="oo")
                    nc.vector.tensor_copy(out=oo[:], in_=hm2[:, 0:2])
                    nc.sync.dma_start(out=h_out[w * 128:(w + 1) * 128, :],
                                      in_=oo[:])
                    continue
                # FF head
                ident = consts["ident"]
                pf = pp.tile([cfg.F2, 128], BF16, tag="pf")
                nc.tensor.transpose(out=pf[:], in_=h2t[:], identity=ident[:])
                t1 = wp.tile([cfg.F2, 128], BF16, tag="t1")
                nc.vector.tensor_copy(out=t1[:], in_=pf[:])
                pm1 = pp.tile([128, cfg.F2 // 2], F32, tag="pm1")
                nc.tensor.matmul(out=pm1[:], lhsT=t1[:], rhs=consts["fW1b"][:],
                                 start=True, stop=True)
                fo = wp.tile([128, cfg.F2 // 2], F32, tag="fo")
                nc.vector.tensor_tensor(out=fo[:], in0=pm1[:], in1=consts["fb1r"][:],
                                        op=OP.add)
                f1 = wp.tile([128, cfg.F2 // 2], BF16, tag="f1")
                nc.scalar.activation(out=f1[:], in_=fo[:], func=AF.Relu)
                pf2 = pp.tile([cfg.F2 // 2, 128], BF16, tag="pf2")
                nc.tensor.transpose(out=pf2[:], in_=f1[:], identity=ident[:])
                t2 = wp.tile([cfg.F2 // 2, 128], BF16, tag="t2")
                nc.vector.tensor_copy(out=t2[:], in_=pf2[:])
                pm2 = pp.tile([128, 2], F32, tag="pm2")
                nc.tensor.matmul(out=pm2[:], lhsT=t2[:], rhs=consts["fW2b"][:],
                                 start=True, stop=True)
                oo = wp.tile([128, 2], F32, tag="oo")
                nc.vector.tensor_tensor(out=oo[:], in0=pm2[:], in1=consts["fb2r"][:],
                                        op=OP.add)
                nc.sync.dma_start(out=h_out[w * 128:(w + 1) * 128, :], in_=oo[:])


# --------------------------------------------------------------------------
# entry points
# --------------------------------------------------------------------------

_INPUT_KEYS = ("x", "W1", "b1", "att_src1", "att_dst1", "W2", "b2",
               "att_src2", "att_dst2", "ffW1", "ffb1", "ffW2", "ffb2")


def _in_maps(plan, inputs):
    cfg = plan["cfg"]
    f32 = lambda a: np.ascontiguousarray(np.asarray(a, np.float32))
    base = {k: f32(inputs[k]) for k in _INPUT_KEYS}
    maps = []
    for c in range(cfg.CORES):
        m = dict(base)
        m["idx1"] = np.ascontiguousarray(plan["idx1"][c])
        m["idx2"] = np.ascontiguousarray(plan["idx2"][c])
        maps.append(m)
    return maps


def _assemble(plan, results):
    cfg = plan["cfg"]
    outs = np.stack([results[c]["out"] for c in range(cfg.CORES)])
    return outs[plan["core2"], plan["q2"]].astype(np.float32)


def kernel(**inputs):
    """Full-input GAT kernel on 8 NeuronCores; returns [50000, 2] float32."""
    cfg = Cfg()
    plan = make_plan(cfg, inputs["edge_index"])
    nc = build_program(plan)
    res = run_bass_kernel_spmd(nc, _in_maps(plan, inputs),
                               list(range(cfg.CORES)))
    return _assemble(plan, res.results)


def kernel_traced(**inputs):
    """kernel() + an HW-exec-time estimate in ns.

    No NTFF profiling is available in this container, so the estimate is a
    differential: amortized wall time (min-of-reps, device-resident inputs)
    of the real NEFF minus the same measurement for a trivial NEFF with an
    identical I/O signature (isolating device execution from dispatch
    overhead).
    """
    import time as _time
    import jax
    cfg = Cfg()
    plan = make_plan(cfg, inputs["edge_index"])
    nc = build_program(plan)
    maps = _in_maps(plan, inputs)
    res = run_bass_kernel_spmd(nc, maps, list(range(cfg.CORES)))
    out = _assemble(plan, res.results)

    exec_ns = None
    try:
        from concourse.bass2jax import (_bass_exec_p, install_neuronx_cc_hook,
                                        partition_id_tensor)
        from jax.sharding import Mesh, PartitionSpec
        from jax.experimental.shard_map import shard_map

        def make_exec(nc_):
            partition_name = (nc_.partition_id_tensor.name
                              if nc_.partition_id_tensor else None)
            in_names, out_names, out_avals, zero_outs = [], [], [], []
            for alloc in nc_.m.functions[0].allocations:
                if not isinstance(alloc, mybir.MemoryLocationSet):
                    continue
                name = alloc.memorylocations[0].name
                if alloc.kind == "ExternalInput":
                    if name != partition_name:
                        in_names.append(name)
                elif alloc.kind == "ExternalOutput":
                    shape = tuple(alloc.tensor_shape)
                    dtype = mybir.dt.np(alloc.dtype)
                    out_names.append(name)
                    out_avals.append(jax.core.ShapedArray(shape, dtype))
                    zero_outs.append(np.zeros(shape, dtype))

            def _body(*args):
                operands = list(args)
                if partition_name is not None:
                    operands.append(partition_id_tensor())
                return tuple(_bass_exec_p.bind(
                    *operands, out_avals=tuple(out_avals),
                    in_names=tuple(in_names + out_names +
                                   ([partition_name] if partition_name else [])),
                    out_names=tuple(out_names),
                    lowering_input_output_aliases=(),
                    sim_require_finite=True, sim_require_nnan=True, nc=nc_))

            devices = jax.devices()[:cfg.CORES]
            mesh = Mesh(np.asarray(devices), ("core",))
            nin = len(in_names) + len(zero_outs)
            fn = jax.jit(shard_map(_body, mesh=mesh,
                                   in_specs=(PartitionSpec("core"),) * nin,
                                   out_specs=(PartitionSpec("core"),) * len(out_names),
                                   check_rep=False), keep_unused=True)
            cat = [np.concatenate([np.asarray(maps[c][nm])
                                   for c in range(cfg.CORES)], axis=0)
                   for nm in in_names]
            cat += [np.zeros((cfg.CORES * z.shape[0], *z.shape[1:]), z.dtype)
                    for z in zero_outs]
            # device-resident inputs: keeps per-call input shipping out of
            # the measurement (it used to dominate and corrupt the diff)
            from jax.sharding import NamedSharding
            sh = NamedSharding(mesh, PartitionSpec("core"))
            dargs = [jax.device_put(a, sh) for a in cat]
            jax.block_until_ready(dargs)
            return fn, dargs

        def rep(fn, args, iters=20):
            t0 = _time.perf_counter()
            os_ = [fn(*args) for _ in range(iters)]
            jax.block_until_ready(os_)
            return (_time.perf_counter() - t0) / iters

        install_neuronx_cc_hook()
        fn, args = make_exec(nc)
        nc2 = _floor_program(plan)
        fn2, args2 = make_exec(nc2)
        # warmup both
        jax.block_until_ready(fn(*args))
        jax.block_until_ready(fn2(*args2))
        # interleave kernel/floor reps so dispatch-overhead drift cancels
        # in the differential; use min-of-reps for each
        tk, tf = [], []
        for _ in range(6):
            tk.append(rep(fn, args))
            tf.append(rep(fn2, args2))
        exec_ns = max(0.0, (min(tk) - min(tf))) * 1e9
    except Exception as e:
        print("timing estimate failed:", str(e)[:200])
    return out, (int(exec_ns) if exec_ns is not None else None)


def _floor_program(plan):
    """Trivial NEFF with identical I/O signature, for differential timing."""
    cfg = plan["cfg"]
    C1 = idx_cols(plan["SLO1"], plan["SHI1"])
    C2 = idx_cols(plan["SLO2"], plan["SHI2"])
    nc = Bacc("TRN2", target_bir_lowering=False, debug=False,
              num_devices=cfg.CORES)
    shapes = dict(x=[cfg.N, cfg.IN], W1=[cfg.IN, 96], b1=[96],
                  att_src1=[3, 32], att_dst1=[3, 32], W2=[96, 96], b2=[32],
                  att_src2=[3, 32], att_dst2=[3, 32], ffW1=[32, 16],
                  ffb1=[16], ffW2=[16, 2], ffb2=[2])
    for k, s in shapes.items():
        nc.dram_tensor(k, s, F32, kind="ExternalInput")
    nc.dram_tensor("idx1", [128, C1], I16, kind="ExternalInput")
    nc.dram_tensor("idx2", [128, C2], I16, kind="ExternalInput")
    t_o = nc.dram_tensor("out", [cfg.NPC, 2], F32, kind="ExternalOutput")
    with tile.TileContext(nc, num_cores=cfg.CORES) as tc:
        with tc.tile_pool(name="p", bufs=1) as p:
            ta = p.tile([128, 2], F32)
            nc.vector.memset(ta[:], 1.0)
            nc.sync.dma_start(out=t_o[0:128, :], in_=ta[:])
    nc.compile()
    return nc

